# revision 17
# baseline (speedup 1.0000x reference)
"""Bass/Trainium2 kernel for ComplexUpSampling2D (2x bilinear, half-pixel centers).

Input:  (16, 128, 128, 128) f32  (B, H, W, C)
Output: (16, 256, 256, 128) f32

Math (per axis, factor 2, half-pixel, with edge clamp):
  out[2i]   = 0.25*in[i-1] + 0.75*in[i]    (in[-1] clamped to in[0])
  out[2i+1] = 0.75*in[i]   + 0.25*in[i+1]  (in[n] clamped to in[n-1])

Strategy (pure data-parallel over batch: 2 images per core on 8 cores):
  - SBUF layout: partitions = H (128), free dim = W*C (16384) per image,
    processed in free-dim chunks of F with a one-w-block halo each side.
  - H-interp mixes partitions -> partition-shifted copies (prv/nxt) of the
    raw chunk via small SBUF->SBUF DMAs on gpsimd (SWDGE).
  - W-interp mixes w-neighbors C elements apart in the free dim -> plain
    shifted access patterns on the halo'd tiles.
  - All weighted sums are single fused scalar_tensor_tensor DVE ops with
    fp32-exact weights:
        curq3 = cur * (3/16)                      (scalar engine)
        qE    = prv * (1/16) + curq3              -> out row 2p   (= row/4)
        qO    = nxt * (1/16) + curq3              -> out row 2p+1
        out[., even w] = 3*q[j] + q[j-1]
        out[., odd  w] = 3*q[j] + q[j+1]
  - Raw bass with explicit semaphores (the walrus codegen on this run path
    supports only one embedded sync-wait per instruction, so all waits are
    standalone wait_ge ops).
  - DMA semaphores are parity-split so that every wait threshold equals
    16 x (all DMAs ever issued on that semaphore at that point): a DMA's
    16 completion increments are spread across engines, so a shared-sem
    cumulative wait can otherwise be satisfied by partial credit from a
    later in-flight DMA.
  - All semaphores are reset to zero at the end behind a finish barrier so
    the NEFF can be re-executed.
"""

from contextlib import ExitStack

import numpy as np

import concourse.bass as bass
from concourse import mybir
from concourse.bass_utils import run_bass_kernel_spmd

B, H, W, C = 16, 128, 128, 128
NCORES = 8
BS = B // NCORES          # images per core
WC = W * C                # 16384 free elements per input row
F = 2048                  # chunk width (input free elements) = 16 w-blocks
NW = F // C               # w-blocks per chunk
NCH = WC // F             # chunks per image
TOT = BS * NCH            # chunks per core
EXT = F + 2 * C           # chunk + one w-block halo on each side
NBUF = 2                  # double buffering (parity sems assume NBUF == 2)

_FP = mybir.dt.float32
_MUL = mybir.AluOpType.mult
_ADD = mybir.AluOpType.add


def _chunks():
    return [(b * NCH + k, b, k) for b in range(BS) for k in range(NCH)]


def _n_in_dmas(k):
    return 2 if (k == 0 or k == NCH - 1) else 1


def _build(**bass_kwargs):
    nc = bass.Bass(**bass_kwargs)
    x = nc.dram_tensor("x", [BS, H, WC], _FP, kind="ExternalInput")
    y = nc.dram_tensor("y", [BS, 2 * H, 2 * WC], _FP, kind="ExternalOutput")

    chunks = _chunks()
    # per-parity cumulative in-DMA counts AFTER chunk ci
    in_par = [0, 0]
    in_cum_par = []     # value of in_par[ci % 2] after chunk ci's loads
    sh_par = [0, 0]
    sh_cum_par = []     # shifted prv/nxt loads: 2 DMAs per load_ext call
    for ci, b, k in chunks:
        in_par[ci % 2] += _n_in_dmas(k)
        in_cum_par.append(in_par[ci % 2])
        sh_par[ci % 2] += 4 * _n_in_dmas(k)
        sh_cum_par.append(sh_par[ci % 2])

    def out_cum(ci):    # store DMAs on parity sem after chunk ci: 2 per chunk
        return 2 * (ci // 2 + 1)

    with ExitStack() as ctx:
        def sb(nm, wide=False):
            return ctx.enter_context(
                nc.sbuf_tensor(nm, [128, 2 * F if wide else EXT], _FP)
            )

        cur = [sb(f"cur{i}") for i in range(NBUF)]
        curq = [sb(f"curq{i}") for i in range(NBUF)]
        prv = [sb(f"prv{i}") for i in range(NBUF)]
        nxt = [sb(f"nxt{i}") for i in range(NBUF)]
        qe = [sb(f"qe{i}") for i in range(NBUF)]
        qo = [sb(f"qo{i}") for i in range(NBUF)]
        oute = [sb(f"oute{i}", wide=True) for i in range(NBUF)]
        outo = [sb(f"outo{i}", wide=True) for i in range(NBUF)]

        sem = lambda nm: ctx.enter_context(nc.semaphore(nm))
        s_in = [sem("s_in0"), sem("s_in1")]
        s_sh = [sem("s_sh0"), sem("s_sh1")]
        s_out = [sem("s_out0"), sem("s_out1")]
        s_act = sem("s_act")
        s_dve = sem("s_dve")
        s_fin = sem("s_fin")
        all_sems = s_in + s_sh + s_out + [s_act, s_dve, s_fin]

        block = ctx.enter_context(nc.Block())

        def load_ext(eng, sem, dst, xb, k, rows_dst, rows_src):
            """DMA a halo'd col-chunk of rows rows_src of xb into dst[rows_dst]."""
            lo = k * F - C
            if k == 0:
                eng.dma_start(out=dst[rows_dst, C:EXT], in_=xb[rows_src, 0 : F + C]).then_inc(sem, 16)
                eng.dma_start(out=dst[rows_dst, 0:C], in_=xb[rows_src, 0:C]).then_inc(sem, 16)
            elif k == NCH - 1:
                eng.dma_start(out=dst[rows_dst, 0 : F + C], in_=xb[rows_src, lo:WC]).then_inc(sem, 16)
                eng.dma_start(out=dst[rows_dst, F + C : EXT], in_=xb[rows_src, WC - C : WC]).then_inc(sem, 16)
            else:
                eng.dma_start(out=dst[rows_dst, :], in_=xb[rows_src, lo : lo + EXT]).then_inc(sem, 16)

        @block.sync
        def _(sync):
            for ci, b, k in chunks:
                p = ci % 2
                if ci >= NBUF:
                    # cur[p]'s only reader is the ACT scale of chunk ci-2
                    sync.wait_ge(s_act, ci - 1)
                    # prv/nxt[p] readers (qE,qO of chunk ci-2) must be done
                    sync.wait_ge(s_dve, 6 * (ci - 2) + 2)
                xb = x[b]
                lo = k * F - C
                if k == 0:
                    sync.dma_start(out=cur[p][:, C:EXT], in_=xb[:, 0 : F + C]).then_inc(s_in[p], 16)
                    sync.dma_start(out=cur[p][:, 0:C], in_=xb[:, 0:C]).then_inc(s_in[p], 16)
                elif k == NCH - 1:
                    sync.dma_start(out=cur[p][:, 0 : F + C], in_=xb[:, lo:WC]).then_inc(s_in[p], 16)
                    sync.dma_start(out=cur[p][:, F + C : EXT], in_=xb[:, WC - C : WC]).then_inc(s_in[p], 16)
                else:
                    sync.dma_start(out=cur[p][:, :], in_=xb[:, lo : lo + EXT]).then_inc(s_in[p], 16)
                # H-shifted copies of the same chunk, straight from DRAM:
                # prv[p] = rows [0, 0..126], nxt[p] = rows [1..127, 127]
                load_ext(sync, s_sh[p], prv[p], xb, k, slice(1, 128), slice(0, 127))
                load_ext(sync, s_sh[p], prv[p], xb, k, slice(0, 1), slice(0, 1))
                load_ext(sync, s_sh[p], nxt[p], xb, k, slice(0, 127), slice(1, 128))
                load_ext(sync, s_sh[p], nxt[p], xb, k, slice(127, 128), slice(127, 128))
                if ci >= 1:
                    pci, pb, pk = chunks[ci - 1]
                    pp = pci % 2
                    sync.wait_ge(s_dve, 6 * pci + 6)
                    yb = y[pb]
                    sync.dma_start(
                        out=yb[0 : 2 * H : 2, 2 * pk * F : 2 * (pk + 1) * F],
                        in_=oute[pp][:],
                    ).then_inc(s_out[pp], 16)
                    sync.dma_start(
                        out=yb[1 : 2 * H : 2, 2 * pk * F : 2 * (pk + 1) * F],
                        in_=outo[pp][:],
                    ).then_inc(s_out[pp], 16)
            # final chunk's stores
            ci, b, k = chunks[-1]
            p = ci % 2
            sync.wait_ge(s_dve, 6 * ci + 6)
            yb = y[b]
            sync.dma_start(
                out=yb[0 : 2 * H : 2, 2 * k * F : 2 * (k + 1) * F], in_=oute[p][:]
            ).then_inc(s_out[p], 16)
            sync.dma_start(
                out=yb[1 : 2 * H : 2, 2 * k * F : 2 * (k + 1) * F], in_=outo[p][:]
            ).then_inc(s_out[p], 16)
            # ---- finish: wait all stores landed, all engines idle, reset sems
            sync.wait_ge(s_out[0], 16 * out_cum(TOT - 2 + (TOT % 2)))
            sync.wait_ge(s_out[1], 16 * out_cum(TOT - 1 - (TOT % 2)))
            sync.wait_ge(s_fin, 2)
            for s in all_sems:
                sync.sem_clear(s)

        @block.scalar
        def _(act):
            for ci, b, k in chunks:
                p = ci % 2
                if ci >= NBUF:
                    # curq[p] readers (qE,qO of chunk ci-2) must be done
                    act.wait_ge(s_dve, 6 * (ci - 2) + 2)
                act.wait_ge(s_in[p], 16 * in_cum_par[ci])
                act.activation(
                    curq[p][:], cur[p][:], mybir.ActivationFunctionType.Copy,
                    scale=0.1875,
                ).then_inc(s_act, 1)
            act.sem_inc(s_fin, 1)

        @block.vector
        def _(vec):
            for ci, b, k in chunks:
                p = ci % 2
                vec.wait_ge(s_act, ci + 1)
                vec.wait_ge(s_sh[p], 16 * sh_cum_par[ci])
                vec.scalar_tensor_tensor(
                    qe[p][:], prv[p][:], 0.0625, curq[p][:], _MUL, _ADD
                ).then_inc(s_dve, 1)
                vec.scalar_tensor_tensor(
                    qo[p][:], nxt[p][:], 0.0625, curq[p][:], _MUL, _ADD
                ).then_inc(s_dve, 1)
                if ci >= NBUF:
                    vec.wait_ge(s_out[p], 16 * out_cum(ci - 2))
                qev = qe[p][:].rearrange("p (a c) -> p a c", c=C)
                qov = qo[p][:].rearrange("p (a c) -> p a c", c=C)
                ev = oute[p][:].rearrange("p (a t c) -> p a t c", t=2, c=C)
                ov = outo[p][:].rearrange("p (a t c) -> p a t c", t=2, c=C)
                vec.scalar_tensor_tensor(
                    ev[:, :, 0, :], qev[:, 1 : NW + 1, :], 3.0,
                    qev[:, 0:NW, :], _MUL, _ADD,
                ).then_inc(s_dve, 1)
                vec.scalar_tensor_tensor(
                    ev[:, :, 1, :], qev[:, 1 : NW + 1, :], 3.0,
                    qev[:, 2 : NW + 2, :], _MUL, _ADD,
                ).then_inc(s_dve, 1)
                vec.scalar_tensor_tensor(
                    ov[:, :, 0, :], qov[:, 1 : NW + 1, :], 3.0,
                    qov[:, 0:NW, :], _MUL, _ADD,
                ).then_inc(s_dve, 1)
                vec.scalar_tensor_tensor(
                    ov[:, :, 1, :], qov[:, 1 : NW + 1, :], 3.0,
                    qov[:, 2 : NW + 2, :], _MUL, _ADD,
                ).then_inc(s_dve, 1)
            vec.sem_inc(s_fin, 1)

    return nc


_NC = None


def kernel(inputs: np.ndarray) -> np.ndarray:
    global _NC
    assert inputs.shape == (B, H, W, C), inputs.shape
    x = np.ascontiguousarray(inputs, dtype=np.float32).reshape(B, H, WC)
    if _NC is None:
        _NC = _build()
    in_maps = [{"x": x[i * BS : (i + 1) * BS]} for i in range(NCORES)]
    res = run_bass_kernel_spmd(_NC, in_maps, list(range(NCORES))).results
    out = np.empty((B, 2 * H, 2 * W, C), dtype=np.float32)
    for i in range(NCORES):
        out[i * BS : (i + 1) * BS] = res[i]["y"].reshape(BS, 2 * H, 2 * W, C)
    return out


# revision 18
# speedup vs baseline: 4.5028x; 4.5028x over previous
"""Bass/Trainium2 kernel for ComplexUpSampling2D (2x bilinear, half-pixel centers).

Input:  (16, 128, 128, 128) f32  (B, H, W, C)
Output: (16, 256, 256, 128) f32

Math (per axis, factor 2, half-pixel, with edge clamp):
  out[2i]   = 0.25*in[i-1] + 0.75*in[i]    (in[-1] clamped to in[0])
  out[2i+1] = 0.75*in[i]   + 0.25*in[i+1]  (in[n] clamped to in[n-1])

Strategy (pure data-parallel over batch: 2 images per core on 8 cores):
  - SBUF layout: partitions = H (128), free dim = W*C (16384) per image,
    processed in free-dim chunks of F with a one-w-block halo each side.
  - H-interp mixes partitions -> done on the TensorEngine as qE = M_E @ cur,
    qO = M_O @ cur with banded 128x128 fp32 matrices (two nonzeros per row:
    3/16 and 1/16, edge rows 4/16) that also fold in the /16 normalization
    and the edge clamp. This avoids partition-shifted DMAs entirely (both
    DGE paths degenerate to one descriptor per partition on one engine for
    partition-misaligned transfers).
  - PSUM results are copied to SBUF by the scalar engine (DMA cannot read
    PSUM, and the W-stage reads each q twice so it cannot stay in PSUM).
  - W-interp mixes w-neighbors C elements apart in the free dim -> fused
    scalar_tensor_tensor DVE ops on shifted access patterns (q = row/4):
        out[., even w] = 3*q[j] + q[j-1]
        out[., odd  w] = 3*q[j] + q[j+1]
  - Raw bass with explicit standalone wait_ge ops (the walrus codegen on
    this run path supports only one embedded sync-wait per instruction).
  - DMA semaphores are parity-split so that every wait threshold equals
    16 x (all DMAs ever issued on that semaphore at that point): a DMA's
    16 completion increments are spread across engines, so a shared-sem
    cumulative wait can otherwise be satisfied by partial credit from a
    later in-flight DMA.
  - All semaphores are reset to zero at the end behind a finish barrier so
    the NEFF can be re-executed.
"""

from contextlib import ExitStack

import numpy as np

import concourse.bass as bass
from concourse import mybir
from concourse.bass_utils import run_bass_kernel_spmd

B, H, W, C = 16, 128, 128, 128
NCORES = 8
BS = B // NCORES          # images per core
WC = W * C                # 16384 free elements per input row
F = 1024                  # chunk width (input free elements) = 8 w-blocks
NW = F // C               # w-blocks per chunk
NCH = WC // F             # chunks per image
TOT = BS * NCH            # chunks per core
EXT = F + 2 * C           # chunk + one w-block halo on each side
NBUF = 2                  # double buffering (parity sems assume NBUF == 2)
MMF = 512                 # max fp32 matmul moving free dim (one PSUM bank)

_FP = mybir.dt.float32
_MUL = mybir.AluOpType.mult
_ADD = mybir.AluOpType.add


def _chunks():
    return [(b * NCH + k, b, k) for b in range(BS) for k in range(NCH)]


def _n_in_dmas(k):
    return 2 if (k == 0 or k == NCH - 1) else 1


def h_weights():
    """lhsT (stationary, [K=in_row, M=out_partition]) for the two H phases."""
    we = np.zeros((H, H), dtype=np.float32)   # qE[m] = out row 2m, = row/4
    i = np.arange(H)
    we[i, i] = 0.1875                          # 3/16
    we[0, 0] = 0.25                            # edge clamp: 4/16
    we[i[:-1], i[:-1] + 1] = 0.0625            # cur[m-1] term: k == m-1
    wo = np.zeros((H, H), dtype=np.float32)   # qO[m] = out row 2m+1
    wo[i, i] = 0.1875
    wo[H - 1, H - 1] = 0.25
    wo[i[1:], i[1:] - 1] = 0.0625              # cur[m+1] term: k == m+1
    return we, wo


def _mm_pieces():
    """(c0, c1) col pieces of EXT, each within one PSUM bank."""
    out = []
    c = 0
    while c < EXT:
        out.append((c, min(c + MMF, EXT)))
        c += MMF
    return out


def _build(**bass_kwargs):
    nc = bass.Bass(**bass_kwargs)
    x = nc.dram_tensor("x", [BS, H, WC], _FP, kind="ExternalInput")
    we_d = nc.dram_tensor("we", [H, H], _FP, kind="ExternalInput")
    wo_d = nc.dram_tensor("wo", [H, H], _FP, kind="ExternalInput")
    y = nc.dram_tensor("y", [BS, 2 * H, 2 * WC], _FP, kind="ExternalOutput")

    chunks = _chunks()
    pieces = _mm_pieces()
    NMM = len(pieces)           # matmuls per phase per chunk
    in_par = [0, 0]
    in_cum_par = []             # per-parity in-DMA count AFTER chunk ci
    for ci, b, k in chunks:
        in_par[ci % 2] += _n_in_dmas(k)
        in_cum_par.append(in_par[ci % 2])

    def out_cum(ci):            # store DMAs on parity sem after chunk ci
        return 2 * (ci // 2 + 1)

    with ExitStack() as ctx:
        def sb(nm, width):
            return ctx.enter_context(nc.sbuf_tensor(nm, [128, width], _FP))

        cur = [sb(f"cur{i}", EXT) for i in range(NBUF)]
        qe = [sb(f"qe{i}", EXT) for i in range(NBUF)]
        qo = [sb(f"qo{i}", EXT) for i in range(NBUF)]
        oute = [sb(f"oute{i}", 2 * F) for i in range(NBUF)]
        outo = [sb(f"outo{i}", 2 * F) for i in range(NBUF)]
        we_sb = sb("we_sb", H)
        wo_sb = sb("wo_sb", H)
        qe_ps = ctx.enter_context(nc.psum_tensor("qe_ps", [128, EXT], _FP))
        qo_ps = ctx.enter_context(nc.psum_tensor("qo_ps", [128, EXT], _FP))

        sem = lambda nm: ctx.enter_context(nc.semaphore(nm))
        s_in = [sem("s_in0"), sem("s_in1")]
        s_out = [sem("s_out0"), sem("s_out1")]
        s_w = sem("s_w")
        s_pe = sem("s_pe")
        s_cp = sem("s_cp")
        s_dve = sem("s_dve")
        s_fin = sem("s_fin")
        all_sems = s_in + s_out + [s_w, s_pe, s_cp, s_dve, s_fin]

        block = ctx.enter_context(nc.Block())

        @block.sync
        def _(sync):
            sync.dma_start(out=we_sb[:], in_=we_d[:]).then_inc(s_w, 16)
            sync.dma_start(out=wo_sb[:], in_=wo_d[:]).then_inc(s_w, 16)
            for ci, b, k in chunks:
                p = ci % 2
                if ci >= NBUF:
                    # cur[p] readers: the 2*NMM matmuls of chunk ci-2
                    sync.wait_ge(s_pe, 2 * NMM * (ci - 1))
                xb = x[b]
                lo = k * F - C
                if k == 0:
                    sync.dma_start(out=cur[p][:, C:EXT], in_=xb[:, 0 : F + C]).then_inc(s_in[p], 16)
                    sync.dma_start(out=cur[p][:, 0:C], in_=xb[:, 0:C]).then_inc(s_in[p], 16)
                elif k == NCH - 1:
                    sync.dma_start(out=cur[p][:, 0 : F + C], in_=xb[:, lo:WC]).then_inc(s_in[p], 16)
                    sync.dma_start(out=cur[p][:, F + C : EXT], in_=xb[:, WC - C : WC]).then_inc(s_in[p], 16)
                else:
                    sync.dma_start(out=cur[p][:, :], in_=xb[:, lo : lo + EXT]).then_inc(s_in[p], 16)
                if ci >= 1:
                    pci, pb, pk = chunks[ci - 1]
                    pp = pci % 2
                    sync.wait_ge(s_dve, 4 * pci + 4)
                    yb = y[pb]
                    sync.dma_start(
                        out=yb[0 : 2 * H : 2, 2 * pk * F : 2 * (pk + 1) * F],
                        in_=oute[pp][:],
                    ).then_inc(s_out[pp], 16)
                    sync.dma_start(
                        out=yb[1 : 2 * H : 2, 2 * pk * F : 2 * (pk + 1) * F],
                        in_=outo[pp][:],
                    ).then_inc(s_out[pp], 16)
            ci, b, k = chunks[-1]
            p = ci % 2
            sync.wait_ge(s_dve, 4 * ci + 4)
            yb = y[b]
            sync.dma_start(
                out=yb[0 : 2 * H : 2, 2 * k * F : 2 * (k + 1) * F], in_=oute[p][:]
            ).then_inc(s_out[p], 16)
            sync.dma_start(
                out=yb[1 : 2 * H : 2, 2 * k * F : 2 * (k + 1) * F], in_=outo[p][:]
            ).then_inc(s_out[p], 16)
            # ---- finish: all stores landed, all engines idle, reset sems
            sync.wait_ge(s_out[0], 16 * out_cum(TOT - 2 + (TOT % 2)))
            sync.wait_ge(s_out[1], 16 * out_cum(TOT - 1 - (TOT % 2)))
            sync.wait_ge(s_fin, 3)
            for s in all_sems:
                sync.sem_clear(s)

        @block.tensor
        def _(pe):
            pe.wait_ge(s_w, 32)
            for ci, b, k in chunks:
                p = ci % 2
                pe.wait_ge(s_in[p], 16 * in_cum_par[ci])
                if ci >= 1:
                    # qe_ps reader (ACT E-copy of chunk ci-1) must be done
                    pe.wait_ge(s_cp, 2 * (ci - 1) + 1)
                for c0, c1 in pieces:
                    pe.matmul(
                        out=qe_ps[:, c0:c1], lhsT=we_sb[:], rhs=cur[p][:, c0:c1],
                        start=True, stop=True,
                    ).then_inc(s_pe, 1)
                if ci >= 1:
                    pe.wait_ge(s_cp, 2 * (ci - 1) + 2)
                for c0, c1 in pieces:
                    pe.matmul(
                        out=qo_ps[:, c0:c1], lhsT=wo_sb[:], rhs=cur[p][:, c0:c1],
                        start=True, stop=True,
                    ).then_inc(s_pe, 1)
            pe.sem_inc(s_fin, 1)

        @block.scalar
        def _(act):
            for ci, b, k in chunks:
                p = ci % 2
                act.wait_ge(s_pe, 2 * NMM * ci + NMM)
                if ci >= NBUF:
                    # qe[p] readers (DVE ops 1,2 of chunk ci-2) must be done
                    act.wait_ge(s_dve, 4 * (ci - 2) + 2)
                act.activation(
                    qe[p][:], qe_ps[:, :], mybir.ActivationFunctionType.Copy,
                ).then_inc(s_cp, 1)
                act.wait_ge(s_pe, 2 * NMM * ci + 2 * NMM)
                if ci >= NBUF:
                    act.wait_ge(s_dve, 4 * (ci - 2) + 4)
                act.activation(
                    qo[p][:], qo_ps[:, :], mybir.ActivationFunctionType.Copy,
                ).then_inc(s_cp, 1)
            act.sem_inc(s_fin, 1)

        @block.vector
        def _(vec):
            for ci, b, k in chunks:
                p = ci % 2
                vec.wait_ge(s_cp, 2 * ci + 1)
                if ci >= NBUF:
                    vec.wait_ge(s_out[p], 16 * out_cum(ci - 2))
                qev = qe[p][:].rearrange("p (a c) -> p a c", c=C)
                qov = qo[p][:].rearrange("p (a c) -> p a c", c=C)
                ev = oute[p][:].rearrange("p (a t c) -> p a t c", t=2, c=C)
                ov = outo[p][:].rearrange("p (a t c) -> p a t c", t=2, c=C)
                vec.scalar_tensor_tensor(
                    ev[:, :, 0, :], qev[:, 1 : NW + 1, :], 3.0,
                    qev[:, 0:NW, :], _MUL, _ADD,
                ).then_inc(s_dve, 1)
                vec.scalar_tensor_tensor(
                    ev[:, :, 1, :], qev[:, 1 : NW + 1, :], 3.0,
                    qev[:, 2 : NW + 2, :], _MUL, _ADD,
                ).then_inc(s_dve, 1)
                vec.wait_ge(s_cp, 2 * ci + 2)
                vec.scalar_tensor_tensor(
                    ov[:, :, 0, :], qov[:, 1 : NW + 1, :], 3.0,
                    qov[:, 0:NW, :], _MUL, _ADD,
                ).then_inc(s_dve, 1)
                vec.scalar_tensor_tensor(
                    ov[:, :, 1, :], qov[:, 1 : NW + 1, :], 3.0,
                    qov[:, 2 : NW + 2, :], _MUL, _ADD,
                ).then_inc(s_dve, 1)
            vec.sem_inc(s_fin, 1)

    return nc


_NC = None


def kernel(inputs: np.ndarray) -> np.ndarray:
    global _NC
    assert inputs.shape == (B, H, W, C), inputs.shape
    x = np.ascontiguousarray(inputs, dtype=np.float32).reshape(B, H, WC)
    if _NC is None:
        _NC = _build()
    we, wo = h_weights()
    in_maps = [
        {"x": x[i * BS : (i + 1) * BS], "we": we, "wo": wo} for i in range(NCORES)
    ]
    res = run_bass_kernel_spmd(_NC, in_maps, list(range(NCORES))).results
    out = np.empty((B, 2 * H, 2 * W, C), dtype=np.float32)
    for i in range(NCORES):
        out[i * BS : (i + 1) * BS] = res[i]["y"].reshape(BS, 2 * H, 2 * W, C)
    return out


# revision 19
# speedup vs baseline: 6.0170x; 1.3363x over previous
"""Bass/Trainium2 kernel for ComplexUpSampling2D (2x bilinear, half-pixel centers).

Input:  (16, 128, 128, 128) f32  (B, H, W, C)
Output: (16, 256, 256, 128) f32

Math (per axis, factor 2, half-pixel, with edge clamp):
  out[2i]   = 0.25*in[i-1] + 0.75*in[i]    (in[-1] clamped to in[0])
  out[2i+1] = 0.75*in[i]   + 0.25*in[i+1]  (in[n] clamped to in[n-1])

Strategy (pure data-parallel over batch: 2 images per core on 8 cores):
  - SBUF layout: partitions = H (128), free dim = W*C (16384) per image,
    processed in free-dim chunks of F with a one-w-block halo each side.
  - H-interp mixes partitions -> done on the TensorEngine as qE = M_E @ cur,
    qO = M_O @ cur with banded 128x128 fp32 matrices (two nonzeros per row:
    3/16 and 1/16, edge rows 4/16) that also fold in the /16 normalization
    and the edge clamp. This avoids partition-shifted DMAs entirely (both
    DGE paths degenerate to one-descriptor-per-partition on a single DMA
    engine for partition-misaligned transfers).
  - PSUM results are copied to SBUF by the scalar engine (DMA cannot read
    PSUM, and the W-stage reads each q twice so it cannot stay in PSUM).
  - W-interp mixes w-neighbors C elements apart in the free dim -> fused
    scalar_tensor_tensor DVE ops on shifted access patterns (q = row/4):
        out[., even w] = 3*q[j] + q[j-1]
        out[., odd  w] = 3*q[j] + q[j+1]
  - Both output row phases are written into one SBUF tile and stored with a
    single DMA per chunk (DRAM rows 2p, 2p+1 are per-partition row pairs).
  - Raw bass with explicit standalone wait_ge ops (the walrus codegen on
    this run path supports only one embedded sync-wait per instruction).
  - DMA semaphores are lane-split (ci % NBUF) so that every wait threshold
    equals 16 x (all DMAs ever issued on that semaphore at that point): a
    DMA's 16 completion increments are spread across engines, so a shared
    cumulative wait could otherwise be satisfied by partial credit from a
    later in-flight DMA on the same semaphore.
  - All semaphores are reset to zero at the end behind a finish barrier so
    the NEFF can be re-executed.
"""

from contextlib import ExitStack

import numpy as np

import concourse.bass as bass
from concourse import mybir
from concourse.bass_utils import run_bass_kernel_spmd

B, H, W, C = 16, 128, 128, 128
NCORES = 8
BS = B // NCORES          # images per core
WC = W * C                # 16384 free elements per input row
F = 1024                  # chunk width (input free elements) = 8 w-blocks
NW = F // C               # w-blocks per chunk
NCH = WC // F             # chunks per image
TOT = BS * NCH            # chunks per core
EXT = F + 2 * C           # chunk + one w-block halo on each side
NBUF = 4                  # buffer depth; lane sems are indexed ci % NBUF
SLAG = 2                  # stores for chunk ci are issued at SP iter ci+SLAG
MMF = 512                 # max fp32 matmul moving free dim (one PSUM bank)

_FP = mybir.dt.float32
_MUL = mybir.AluOpType.mult
_ADD = mybir.AluOpType.add


def _chunks():
    return [(b * NCH + k, b, k) for b in range(BS) for k in range(NCH)]


def _n_in_dmas(k):
    return 2 if (k == 0 or k == NCH - 1) else 1


def h_weights():
    """lhsT (stationary, [K=in_row, M=out_partition]) for the two H phases."""
    we = np.zeros((H, H), dtype=np.float32)   # qE[m] = out row 2m, = row/4
    i = np.arange(H)
    we[i, i] = 0.1875                          # 3/16
    we[0, 0] = 0.25                            # edge clamp: 4/16
    we[i[:-1], i[:-1] + 1] = 0.0625            # cur[m-1] term: k == m-1
    wo = np.zeros((H, H), dtype=np.float32)   # qO[m] = out row 2m+1
    wo[i, i] = 0.1875
    wo[H - 1, H - 1] = 0.25
    wo[i[1:], i[1:] - 1] = 0.0625              # cur[m+1] term: k == m+1
    return we, wo


def _mm_pieces():
    """(c0, c1) col pieces of EXT, each within one PSUM bank."""
    out = []
    c = 0
    while c < EXT:
        out.append((c, min(c + MMF, EXT)))
        c += MMF
    return out


def _build(**bass_kwargs):
    nc = bass.Bass(**bass_kwargs)
    x = nc.dram_tensor("x", [BS, H, WC], _FP, kind="ExternalInput")
    we_d = nc.dram_tensor("we", [H, H], _FP, kind="ExternalInput")
    wo_d = nc.dram_tensor("wo", [H, H], _FP, kind="ExternalInput")
    y = nc.dram_tensor("y", [BS, 2 * H, 2 * WC], _FP, kind="ExternalOutput")

    chunks = _chunks()
    pieces = _mm_pieces()
    NMM = len(pieces)           # matmuls per phase per chunk
    in_lane = [0] * NBUF
    in_cum_lane = []            # per-lane in-DMA count AFTER chunk ci
    for ci, b, k in chunks:
        in_lane[ci % NBUF] += _n_in_dmas(k)
        in_cum_lane.append(in_lane[ci % NBUF])

    def st_cnt(ci):             # store DMAs on lane sem through chunk ci
        return ci // NBUF + 1

    with ExitStack() as ctx:
        def sb(nm, width):
            return ctx.enter_context(nc.sbuf_tensor(nm, [128, width], _FP))

        cur = [sb(f"cur{i}", EXT) for i in range(NBUF)]
        qe = [sb(f"qe{i}", EXT) for i in range(NBUF)]
        qo = [sb(f"qo{i}", EXT) for i in range(NBUF)]
        outt = [sb(f"outt{i}", 4 * F) for i in range(NBUF)]
        we_sb = sb("we_sb", H)
        wo_sb = sb("wo_sb", H)
        # 1536 cols = 3 whole PSUM banks each, so every 512-col matmul piece
        # sits inside a single bank
        qe_ps = ctx.enter_context(nc.psum_tensor("qe_ps", [128, 1536], _FP))
        qo_ps = ctx.enter_context(nc.psum_tensor("qo_ps", [128, 1536], _FP))

        sem = lambda nm: ctx.enter_context(nc.semaphore(nm))
        s_in = [sem(f"s_in{i}") for i in range(NBUF)]
        s_out = [sem(f"s_out{i}") for i in range(NBUF)]
        s_w = sem("s_w")
        s_pe = sem("s_pe")
        s_cp = sem("s_cp")
        s_dve = sem("s_dve")
        s_fin = sem("s_fin")
        all_sems = s_in + s_out + [s_w, s_pe, s_cp, s_dve, s_fin]

        block = ctx.enter_context(nc.Block())

        def store(sync, ci):
            _, b, k = chunks[ci]
            l = ci % NBUF
            sync.wait_ge(s_dve, 4 * ci + 4)
            # partition p -> output rows 2p and 2p+1
            dst = y[b].rearrange("(p t) w -> p t w", t=2)[
                :, :, 2 * k * F : 2 * (k + 1) * F
            ]
            src = outt[l][:].rearrange("p (t w) -> p t w", t=2)
            sync.dma_start(out=dst, in_=src).then_inc(s_out[l], 16)

        @block.sync
        def _(sync):
            sync.dma_start(out=we_sb[:], in_=we_d[:]).then_inc(s_w, 16)
            sync.dma_start(out=wo_sb[:], in_=wo_d[:]).then_inc(s_w, 16)
            for ci, b, k in chunks:
                l = ci % NBUF
                if ci >= NBUF:
                    # cur[l] readers: the 2*NMM matmuls of chunk ci-NBUF
                    sync.wait_ge(s_pe, 2 * NMM * (ci - NBUF + 1))
                xb = x[b]
                lo = k * F - C
                if k == 0:
                    sync.dma_start(out=cur[l][:, C:EXT], in_=xb[:, 0 : F + C]).then_inc(s_in[l], 16)
                    sync.dma_start(out=cur[l][:, 0:C], in_=xb[:, 0:C]).then_inc(s_in[l], 16)
                elif k == NCH - 1:
                    sync.dma_start(out=cur[l][:, 0 : F + C], in_=xb[:, lo:WC]).then_inc(s_in[l], 16)
                    sync.dma_start(out=cur[l][:, F + C : EXT], in_=xb[:, WC - C : WC]).then_inc(s_in[l], 16)
                else:
                    sync.dma_start(out=cur[l][:, :], in_=xb[:, lo : lo + EXT]).then_inc(s_in[l], 16)
                if ci >= SLAG:
                    store(sync, ci - SLAG)
            for ci in range(TOT - SLAG, TOT):
                store(sync, ci)
            # ---- finish: all stores landed, all engines idle, reset sems
            for l in range(NBUF):
                last = TOT - 1 - ((TOT - 1 - l) % NBUF)
                sync.wait_ge(s_out[l], 16 * st_cnt(last))
            sync.wait_ge(s_fin, 3)
            for s in all_sems:
                sync.sem_clear(s)

        @block.tensor
        def _(pe):
            pe.wait_ge(s_w, 32)
            for ci, b, k in chunks:
                l = ci % NBUF
                pe.wait_ge(s_in[l], 16 * in_cum_lane[ci])
                if ci >= 1:
                    # qe_ps reader (ACT E-copy of chunk ci-1) must be done
                    pe.wait_ge(s_cp, 2 * (ci - 1) + 1)
                for c0, c1 in pieces:
                    pe.matmul(
                        out=qe_ps[:, c0:c1], lhsT=we_sb[:], rhs=cur[l][:, c0:c1],
                        start=True, stop=True,
                    ).then_inc(s_pe, 1)
                if ci >= 1:
                    pe.wait_ge(s_cp, 2 * (ci - 1) + 2)
                for c0, c1 in pieces:
                    pe.matmul(
                        out=qo_ps[:, c0:c1], lhsT=wo_sb[:], rhs=cur[l][:, c0:c1],
                        start=True, stop=True,
                    ).then_inc(s_pe, 1)
            pe.sem_inc(s_fin, 1)

        @block.scalar
        def _(act):
            for ci, b, k in chunks:
                l = ci % NBUF
                act.wait_ge(s_pe, 2 * NMM * ci + NMM)
                if ci >= NBUF:
                    # qe[l] readers (DVE ops 1,2 of chunk ci-NBUF) must be done
                    act.wait_ge(s_dve, 4 * (ci - NBUF) + 2)
                act.activation(
                    qe[l][:], qe_ps[:, 0:EXT], mybir.ActivationFunctionType.Copy,
                ).then_inc(s_cp, 1)
                act.wait_ge(s_pe, 2 * NMM * ci + 2 * NMM)
                if ci >= NBUF:
                    act.wait_ge(s_dve, 4 * (ci - NBUF) + 4)
                act.activation(
                    qo[l][:], qo_ps[:, 0:EXT], mybir.ActivationFunctionType.Copy,
                ).then_inc(s_cp, 1)
            act.sem_inc(s_fin, 1)

        @block.vector
        def _(vec):
            for ci, b, k in chunks:
                l = ci % NBUF
                vec.wait_ge(s_cp, 2 * ci + 1)
                if ci >= NBUF:
                    vec.wait_ge(s_out[l], 16 * st_cnt(ci - NBUF))
                qev = qe[l][:].rearrange("p (a c) -> p a c", c=C)
                qov = qo[l][:].rearrange("p (a c) -> p a c", c=C)
                ov = outt[l][:].rearrange("p (t a u c) -> p t a u c", t=2, u=2, c=C)
                vec.scalar_tensor_tensor(
                    ov[:, 0, :, 0, :], qev[:, 1 : NW + 1, :], 3.0,
                    qev[:, 0:NW, :], _MUL, _ADD,
                ).then_inc(s_dve, 1)
                vec.scalar_tensor_tensor(
                    ov[:, 0, :, 1, :], qev[:, 1 : NW + 1, :], 3.0,
                    qev[:, 2 : NW + 2, :], _MUL, _ADD,
                ).then_inc(s_dve, 1)
                vec.wait_ge(s_cp, 2 * ci + 2)
                vec.scalar_tensor_tensor(
                    ov[:, 1, :, 0, :], qov[:, 1 : NW + 1, :], 3.0,
                    qov[:, 0:NW, :], _MUL, _ADD,
                ).then_inc(s_dve, 1)
                vec.scalar_tensor_tensor(
                    ov[:, 1, :, 1, :], qov[:, 1 : NW + 1, :], 3.0,
                    qov[:, 2 : NW + 2, :], _MUL, _ADD,
                ).then_inc(s_dve, 1)
            vec.sem_inc(s_fin, 1)

    return nc


_NC = None


def kernel(inputs: np.ndarray) -> np.ndarray:
    global _NC
    assert inputs.shape == (B, H, W, C), inputs.shape
    x = np.ascontiguousarray(inputs, dtype=np.float32).reshape(B, H, WC)
    if _NC is None:
        _NC = _build()
    we, wo = h_weights()
    in_maps = [
        {"x": x[i * BS : (i + 1) * BS], "we": we, "wo": wo} for i in range(NCORES)
    ]
    res = run_bass_kernel_spmd(_NC, in_maps, list(range(NCORES))).results
    out = np.empty((B, 2 * H, 2 * W, C), dtype=np.float32)
    for i in range(NCORES):
        out[i * BS : (i + 1) * BS] = res[i]["y"].reshape(BS, 2 * H, 2 * W, C)
    return out


# revision 20
# speedup vs baseline: 6.1950x; 1.0296x over previous
"""Bass/Trainium2 kernel for ComplexUpSampling2D (2x bilinear, half-pixel centers).

Input:  (16, 128, 128, 128) f32  (B, H, W, C)
Output: (16, 256, 256, 128) f32

Math (per axis, factor 2, half-pixel, with edge clamp):
  out[2i]   = 0.25*in[i-1] + 0.75*in[i]    (in[-1] clamped to in[0])
  out[2i+1] = 0.75*in[i]   + 0.25*in[i+1]  (in[n] clamped to in[n-1])

Strategy (pure data-parallel over batch: 2 images per core on 8 cores):
  - SBUF layout: partitions = H (128), free dim = W*C (16384) per image.
    Each image is loaded ONCE into a resident tile with a duplicated C-block
    on each end (the W edge clamp), so every F-wide compute chunk slices a
    uniform (F + 2C)-wide halo'd window out of it - no per-chunk input DMAs
    and minimal HBM read traffic.
  - H-interp mixes partitions -> done on the TensorEngine as qE = M_E @ cur,
    qO = M_O @ cur with banded 128x128 fp32 matrices (two nonzeros per row:
    3/16 and 1/16, edge rows 4/16) that also fold in the /16 normalization
    and the H edge clamp. This avoids partition-shifted DMAs entirely (both
    DGE paths degenerate to one-descriptor-per-partition on a single DMA
    engine for partition-misaligned transfers).
  - PSUM results are copied to SBUF by the scalar engine (DMA cannot read
    PSUM, and the W-stage reads each q twice so it cannot stay in PSUM).
  - W-interp mixes w-neighbors C elements apart in the free dim -> fused
    scalar_tensor_tensor DVE ops on shifted access patterns (q = row/4):
        out[., even w] = 3*q[j] + q[j-1]
        out[., odd  w] = 3*q[j] + q[j+1]
  - Both output row phases are written into one SBUF tile and stored with a
    single DMA per chunk (DRAM rows 2p, 2p+1 are per-partition row pairs).
  - Raw bass with explicit standalone wait_ge ops (the walrus codegen on
    this run path supports only one embedded sync-wait per instruction).
  - DMA semaphores are lane-split so that every wait threshold equals
    16 x (all DMAs ever issued on that semaphore at that point): a DMA's 16
    completion increments are spread across engines, so a shared cumulative
    wait could otherwise be satisfied by partial credit from a later
    in-flight DMA on the same semaphore.
  - All semaphores are reset to zero at the end behind a finish barrier so
    the NEFF can be re-executed.
"""

from contextlib import ExitStack

import numpy as np

import concourse.bass as bass
from concourse import mybir
from concourse.bass_utils import run_bass_kernel_spmd

B, H, W, C = 16, 128, 128, 128
NCORES = 8
BS = B // NCORES          # images per core
WC = W * C                # 16384 free elements per input row
F = 1024                  # chunk width (input free elements) = 8 w-blocks
NW = F // C               # w-blocks per chunk
NCH = WC // F             # chunks per image
TOT = BS * NCH            # chunks per core
EXT = F + 2 * C           # chunk + one w-block halo on each side
NBUF = 2                  # buffer depth for q/out tiles; lane sems ci % NBUF
MMF = 512                 # max fp32 matmul moving free dim (one PSUM bank)

_FP = mybir.dt.float32
_MUL = mybir.AluOpType.mult
_ADD = mybir.AluOpType.add


def _chunks():
    return [(b * NCH + k, b, k) for b in range(BS) for k in range(NCH)]


def h_weights():
    """lhsT (stationary, [K=in_row, M=out_partition]) for the two H phases."""
    we = np.zeros((H, H), dtype=np.float32)   # qE[m] = out row 2m, = row/4
    i = np.arange(H)
    we[i, i] = 0.1875                          # 3/16
    we[0, 0] = 0.25                            # edge clamp: 4/16
    we[i[:-1], i[:-1] + 1] = 0.0625            # cur[m-1] term: k == m-1
    wo = np.zeros((H, H), dtype=np.float32)   # qO[m] = out row 2m+1
    wo[i, i] = 0.1875
    wo[H - 1, H - 1] = 0.25
    wo[i[1:], i[1:] - 1] = 0.0625              # cur[m+1] term: k == m+1
    return we, wo


def _mm_pieces():
    """(c0, c1) col pieces of EXT, each within one PSUM bank."""
    out = []
    c = 0
    while c < EXT:
        out.append((c, min(c + MMF, EXT)))
        c += MMF
    return out


def _build(**bass_kwargs):
    nc = bass.Bass(**bass_kwargs)
    x = nc.dram_tensor("x", [BS, H, WC], _FP, kind="ExternalInput")
    we_d = nc.dram_tensor("we", [H, H], _FP, kind="ExternalInput")
    wo_d = nc.dram_tensor("wo", [H, H], _FP, kind="ExternalInput")
    y = nc.dram_tensor("y", [BS, 2 * H, 2 * WC], _FP, kind="ExternalOutput")

    chunks = _chunks()
    pieces = _mm_pieces()
    NMM = len(pieces)           # matmuls per phase per chunk

    def st_cnt(ci):             # store DMAs on lane sem through chunk ci
        return ci // NBUF + 1

    with ExitStack() as ctx:
        def sb(nm, width):
            return ctx.enter_context(nc.sbuf_tensor(nm, [128, width], _FP))

        img = [sb(f"img{i}", 2 * C + WC) for i in range(BS)]
        qe = [sb(f"qe{i}", EXT) for i in range(NBUF)]
        qo = [sb(f"qo{i}", EXT) for i in range(NBUF)]
        outt = [sb(f"outt{i}", 4 * F) for i in range(NBUF)]
        we_sb = sb("we_sb", H)
        wo_sb = sb("wo_sb", H)
        # 1536 cols = 3 whole PSUM banks each, so every 512-col matmul piece
        # sits inside a single bank
        qe_ps = ctx.enter_context(nc.psum_tensor("qe_ps", [128, 1536], _FP))
        qo_ps = ctx.enter_context(nc.psum_tensor("qo_ps", [128, 1536], _FP))

        sem = lambda nm: ctx.enter_context(nc.semaphore(nm))
        s_in = [sem(f"s_in{i}") for i in range(BS)]
        s_out = [sem(f"s_out{i}") for i in range(NBUF)]
        s_w = sem("s_w")
        s_pe = sem("s_pe")
        s_cp = sem("s_cp")
        s_dve = sem("s_dve")
        s_fin = sem("s_fin")
        all_sems = s_in + s_out + [s_w, s_pe, s_cp, s_dve, s_fin]

        block = ctx.enter_context(nc.Block())

        @block.sync
        def _(sync):
            sync.dma_start(out=we_sb[:], in_=we_d[:]).then_inc(s_w, 16)
            sync.dma_start(out=wo_sb[:], in_=wo_d[:]).then_inc(s_w, 16)
            for b in range(BS):
                # whole image + duplicated first/last w-block (W edge clamp)
                sync.dma_start(out=img[b][:, C : C + WC], in_=x[b][:, :]).then_inc(s_in[b], 16)
                sync.dma_start(out=img[b][:, 0:C], in_=x[b][:, 0:C]).then_inc(s_in[b], 16)
                sync.dma_start(out=img[b][:, C + WC :], in_=x[b][:, WC - C : WC]).then_inc(s_in[b], 16)
            for ci, b, k in chunks:
                l = ci % NBUF
                sync.wait_ge(s_dve, 4 * ci + 4)
                # partition p -> output rows 2p and 2p+1
                dst = y[b].rearrange("(p t) w -> p t w", t=2)[
                    :, :, 2 * k * F : 2 * (k + 1) * F
                ]
                src = outt[l][:].rearrange("p (t w) -> p t w", t=2)
                sync.dma_start(out=dst, in_=src).then_inc(s_out[l], 16)
            # ---- finish: all stores landed, all engines idle, reset sems
            for l in range(NBUF):
                last = TOT - 1 - ((TOT - 1 - l) % NBUF)
                sync.wait_ge(s_out[l], 16 * st_cnt(last))
            sync.wait_ge(s_fin, 3)
            for s in all_sems:
                sync.sem_clear(s)

        @block.tensor
        def _(pe):
            pe.wait_ge(s_w, 32)
            for ci, b, k in chunks:
                pe.wait_ge(s_in[b], 48)
                if ci >= 1:
                    # qe_ps reader (ACT E-copy of chunk ci-1) must be done
                    pe.wait_ge(s_cp, 2 * (ci - 1) + 1)
                rhs = img[b][:, k * F : k * F + EXT]
                for c0, c1 in pieces:
                    pe.matmul(
                        out=qe_ps[:, c0:c1], lhsT=we_sb[:], rhs=rhs[:, c0:c1],
                        start=True, stop=True,
                    ).then_inc(s_pe, 1)
                if ci >= 1:
                    pe.wait_ge(s_cp, 2 * (ci - 1) + 2)
                for c0, c1 in pieces:
                    pe.matmul(
                        out=qo_ps[:, c0:c1], lhsT=wo_sb[:], rhs=rhs[:, c0:c1],
                        start=True, stop=True,
                    ).then_inc(s_pe, 1)
            pe.sem_inc(s_fin, 1)

        @block.scalar
        def _(act):
            for ci, b, k in chunks:
                l = ci % NBUF
                act.wait_ge(s_pe, 2 * NMM * ci + NMM)
                if ci >= NBUF:
                    # qe[l] readers (DVE ops 1,2 of chunk ci-NBUF) must be done
                    act.wait_ge(s_dve, 4 * (ci - NBUF) + 2)
                act.activation(
                    qe[l][:], qe_ps[:, 0:EXT], mybir.ActivationFunctionType.Copy,
                ).then_inc(s_cp, 1)
                act.wait_ge(s_pe, 2 * NMM * ci + 2 * NMM)
                if ci >= NBUF:
                    act.wait_ge(s_dve, 4 * (ci - NBUF) + 4)
                act.activation(
                    qo[l][:], qo_ps[:, 0:EXT], mybir.ActivationFunctionType.Copy,
                ).then_inc(s_cp, 1)
            act.sem_inc(s_fin, 1)

        @block.vector
        def _(vec):
            for ci, b, k in chunks:
                l = ci % NBUF
                vec.wait_ge(s_cp, 2 * ci + 1)
                if ci >= NBUF:
                    vec.wait_ge(s_out[l], 16 * st_cnt(ci - NBUF))
                qev = qe[l][:].rearrange("p (a c) -> p a c", c=C)
                qov = qo[l][:].rearrange("p (a c) -> p a c", c=C)
                ov = outt[l][:].rearrange("p (t a u c) -> p t a u c", t=2, u=2, c=C)
                vec.scalar_tensor_tensor(
                    ov[:, 0, :, 0, :], qev[:, 1 : NW + 1, :], 3.0,
                    qev[:, 0:NW, :], _MUL, _ADD,
                ).then_inc(s_dve, 1)
                vec.scalar_tensor_tensor(
                    ov[:, 0, :, 1, :], qev[:, 1 : NW + 1, :], 3.0,
                    qev[:, 2 : NW + 2, :], _MUL, _ADD,
                ).then_inc(s_dve, 1)
                vec.wait_ge(s_cp, 2 * ci + 2)
                vec.scalar_tensor_tensor(
                    ov[:, 1, :, 0, :], qov[:, 1 : NW + 1, :], 3.0,
                    qov[:, 0:NW, :], _MUL, _ADD,
                ).then_inc(s_dve, 1)
                vec.scalar_tensor_tensor(
                    ov[:, 1, :, 1, :], qov[:, 1 : NW + 1, :], 3.0,
                    qov[:, 2 : NW + 2, :], _MUL, _ADD,
                ).then_inc(s_dve, 1)
            vec.sem_inc(s_fin, 1)

    return nc


_NC = None


def kernel(inputs: np.ndarray) -> np.ndarray:
    global _NC
    assert inputs.shape == (B, H, W, C), inputs.shape
    x = np.ascontiguousarray(inputs, dtype=np.float32).reshape(B, H, WC)
    if _NC is None:
        _NC = _build()
    we, wo = h_weights()
    in_maps = [
        {"x": x[i * BS : (i + 1) * BS], "we": we, "wo": wo} for i in range(NCORES)
    ]
    res = run_bass_kernel_spmd(_NC, in_maps, list(range(NCORES))).results
    out = np.empty((B, 2 * H, 2 * W, C), dtype=np.float32)
    for i in range(NCORES):
        out[i * BS : (i + 1) * BS] = res[i]["y"].reshape(BS, 2 * H, 2 * W, C)
    return out


# revision 22
# speedup vs baseline: 6.9256x; 1.1179x over previous
"""Bass/Trainium2 kernel for ComplexUpSampling2D (2x bilinear, half-pixel centers).

Input:  (16, 128, 128, 128) f32  (B, H, W, C)
Output: (16, 256, 256, 128) f32

Math (per axis, factor 2, half-pixel, with edge clamp):
  out[2i]   = 0.25*in[i-1] + 0.75*in[i]    (in[-1] clamped to in[0])
  out[2i+1] = 0.75*in[i]   + 0.25*in[i+1]  (in[n] clamped to in[n-1])

Strategy (pure data-parallel over batch: 2 images per core on 8 cores):
  - SBUF layout: partitions = H (128), free dim = W*C (16384) per image.
    Each image is loaded ONCE into a resident tile with a duplicated C-block
    on each end (the W edge clamp), so every F-wide compute chunk slices a
    uniform (F + 2C)-wide halo'd window out of it - no per-chunk input DMAs
    and minimal HBM read traffic.
  - H-interp mixes partitions -> done on the TensorEngine as qE = M_E @ cur,
    qO = M_O @ cur with banded 128x128 fp32 matrices (two nonzeros per row:
    3/16 and 1/16, edge rows 4/16) that also fold in the /16 normalization
    and the H edge clamp. This avoids partition-shifted DMAs entirely (both
    DGE paths degenerate to one-descriptor-per-partition on a single DMA
    engine for partition-misaligned transfers).
  - PSUM results are copied to SBUF by the scalar engine (DMA cannot read
    PSUM, and the W-stage reads each q twice so it cannot stay in PSUM).
  - W-interp mixes w-neighbors C elements apart in the free dim -> fused
    scalar_tensor_tensor DVE ops on shifted access patterns (q = row/4):
        out[., even w] = 3*q[j] + q[j-1]
        out[., odd  w] = 3*q[j] + q[j+1]
  - Both output row phases are written into one SBUF tile and stored with a
    single DMA per chunk (DRAM rows 2p, 2p+1 are per-partition row pairs).
  - Raw bass with explicit standalone wait_ge ops (the walrus codegen on
    this run path supports only one embedded sync-wait per instruction).
  - DMA semaphores are lane-split so that every wait threshold equals
    16 x (all DMAs ever issued on that semaphore at that point): a DMA's 16
    completion increments are spread across engines, so a shared cumulative
    wait could otherwise be satisfied by partial credit from a later
    in-flight DMA on the same semaphore.
  - All semaphores are reset to zero at the end behind a finish barrier so
    the NEFF can be re-executed.
"""

from contextlib import ExitStack

import numpy as np

import concourse.bass as bass
from concourse import mybir
from concourse.bass_utils import run_bass_kernel_spmd

B, H, W, C = 16, 128, 128, 128
NCORES = 8
BS = B // NCORES          # images per core
WC = W * C                # 16384 free elements per input row
F = 1024                  # chunk width (input free elements) = 8 w-blocks
NW = F // C               # w-blocks per chunk
NCH = WC // F             # chunks per image
TOT = BS * NCH            # chunks per core
EXT = F + 2 * C           # chunk + one w-block halo on each side
NBUF = 2                  # buffer depth for q/out tiles; lane sems ci % NBUF
MMF = 512                 # max fp32 matmul moving free dim (one PSUM bank)

_FP = mybir.dt.float32
_MUL = mybir.AluOpType.mult
_ADD = mybir.AluOpType.add


def _chunks():
    return [(b * NCH + k, b, k) for b in range(BS) for k in range(NCH)]


def h_weights():
    """lhsT (stationary, [K=in_row, M=out_partition]) for the two H phases."""
    we = np.zeros((H, H), dtype=np.float32)   # qE[m] = out row 2m, = row/4
    i = np.arange(H)
    we[i, i] = 0.1875                          # 3/16
    we[0, 0] = 0.25                            # edge clamp: 4/16
    we[i[:-1], i[:-1] + 1] = 0.0625            # cur[m-1] term: k == m-1
    wo = np.zeros((H, H), dtype=np.float32)   # qO[m] = out row 2m+1
    wo[i, i] = 0.1875
    wo[H - 1, H - 1] = 0.25
    wo[i[1:], i[1:] - 1] = 0.0625              # cur[m+1] term: k == m+1
    return we, wo


def _mm_pieces():
    """(c0, c1) col pieces of EXT, each within one PSUM bank."""
    out = []
    c = 0
    while c < EXT:
        out.append((c, min(c + MMF, EXT)))
        c += MMF
    return out


def _build(**bass_kwargs):
    nc = bass.Bass(**bass_kwargs)
    x = nc.dram_tensor("x", [BS, H, WC], _FP, kind="ExternalInput")
    we_d = nc.dram_tensor("we", [H, H], _FP, kind="ExternalInput")
    wo_d = nc.dram_tensor("wo", [H, H], _FP, kind="ExternalInput")
    y = nc.dram_tensor("y", [BS, 2 * H, 2 * WC], _FP, kind="ExternalOutput")

    chunks = _chunks()
    pieces = _mm_pieces()
    NMM = len(pieces)           # matmuls per phase per chunk

    def st_cnt(ci):             # store DMAs on lane sem through chunk ci
        return 2 * (ci // NBUF + 1)

    with ExitStack() as ctx:
        def sb(nm, width):
            return ctx.enter_context(nc.sbuf_tensor(nm, [128, width], _FP))

        img = [sb(f"img{i}", 2 * C + WC) for i in range(BS)]
        qe = [sb(f"qe{i}", EXT) for i in range(NBUF)]
        qo = [sb(f"qo{i}", EXT) for i in range(NBUF)]
        outt = [sb(f"outt{i}", 4 * F) for i in range(NBUF)]
        we_sb = sb("we_sb", H)
        wo_sb = sb("wo_sb", H)
        # 1536 cols = 3 whole PSUM banks each, so every 512-col matmul piece
        # sits inside a single bank
        qe_ps = ctx.enter_context(nc.psum_tensor("qe_ps", [128, 1536], _FP))
        qo_ps = ctx.enter_context(nc.psum_tensor("qo_ps", [128, 1536], _FP))

        sem = lambda nm: ctx.enter_context(nc.semaphore(nm))
        s_in = [sem(f"s_in{i}") for i in range(BS)]
        s_out = [sem(f"s_out{i}") for i in range(NBUF)]
        s_w = sem("s_w")
        s_pe = sem("s_pe")
        s_cp = sem("s_cp")
        s_dve = sem("s_dve")
        s_fin = sem("s_fin")
        all_sems = s_in + s_out + [s_w, s_pe, s_cp, s_dve, s_fin]

        block = ctx.enter_context(nc.Block())

        @block.sync
        def _(sync):
            sync.dma_start(out=we_sb[:], in_=we_d[:]).then_inc(s_w, 16)
            sync.dma_start(out=wo_sb[:], in_=wo_d[:]).then_inc(s_w, 16)
            for b in range(BS):
                # whole image + duplicated first/last w-block (W edge clamp)
                sync.dma_start(out=img[b][:, C : C + WC], in_=x[b][:, :]).then_inc(s_in[b], 16)
                sync.dma_start(out=img[b][:, 0:C], in_=x[b][:, 0:C]).then_inc(s_in[b], 16)
                sync.dma_start(out=img[b][:, C + WC :], in_=x[b][:, WC - C : WC]).then_inc(s_in[b], 16)
            for ci, b, k in chunks:
                l = ci % NBUF
                cols = slice(2 * k * F, 2 * (k + 1) * F)
                # even rows as soon as DVE ops 1,2 are done; odd after 3,4
                sync.wait_ge(s_dve, 4 * ci + 2)
                sync.dma_start(
                    out=y[b][0 : 2 * H : 2, cols], in_=outt[l][:, 0 : 2 * F]
                ).then_inc(s_out[l], 16)
                sync.wait_ge(s_dve, 4 * ci + 4)
                sync.dma_start(
                    out=y[b][1 : 2 * H : 2, cols], in_=outt[l][:, 2 * F : 4 * F]
                ).then_inc(s_out[l], 16)
            # ---- finish: all stores landed, all engines idle, reset sems
            for l in range(NBUF):
                last = TOT - 1 - ((TOT - 1 - l) % NBUF)
                sync.wait_ge(s_out[l], 16 * st_cnt(last))
            sync.wait_ge(s_fin, 3)
            for s in all_sems:
                sync.sem_clear(s)

        @block.tensor
        def _(pe):
            pe.wait_ge(s_w, 32)
            for ci, b, k in chunks:
                pe.wait_ge(s_in[b], 48)
                if ci >= 1:
                    # qe_ps reader (ACT E-copy of chunk ci-1) must be done
                    pe.wait_ge(s_cp, 2 * (ci - 1) + 1)
                rhs = img[b][:, k * F : k * F + EXT]
                for c0, c1 in pieces:
                    pe.matmul(
                        out=qe_ps[:, c0:c1], lhsT=we_sb[:], rhs=rhs[:, c0:c1],
                        start=True, stop=True,
                    ).then_inc(s_pe, 1)
                if ci >= 1:
                    pe.wait_ge(s_cp, 2 * (ci - 1) + 2)
                for c0, c1 in pieces:
                    pe.matmul(
                        out=qo_ps[:, c0:c1], lhsT=wo_sb[:], rhs=rhs[:, c0:c1],
                        start=True, stop=True,
                    ).then_inc(s_pe, 1)
            pe.sem_inc(s_fin, 1)

        @block.scalar
        def _(act):
            for ci, b, k in chunks:
                l = ci % NBUF
                act.wait_ge(s_pe, 2 * NMM * ci + NMM)
                if ci >= NBUF:
                    # qe[l] readers (DVE ops 1,2 of chunk ci-NBUF) must be done
                    act.wait_ge(s_dve, 4 * (ci - NBUF) + 2)
                act.activation(
                    qe[l][:], qe_ps[:, 0:EXT], mybir.ActivationFunctionType.Copy,
                ).then_inc(s_cp, 1)
                act.wait_ge(s_pe, 2 * NMM * ci + 2 * NMM)
                if ci >= NBUF:
                    act.wait_ge(s_dve, 4 * (ci - NBUF) + 4)
                act.activation(
                    qo[l][:], qo_ps[:, 0:EXT], mybir.ActivationFunctionType.Copy,
                ).then_inc(s_cp, 1)
            act.sem_inc(s_fin, 1)

        @block.vector
        def _(vec):
            for ci, b, k in chunks:
                l = ci % NBUF
                vec.wait_ge(s_cp, 2 * ci + 1)
                if ci >= NBUF:
                    vec.wait_ge(s_out[l], 16 * st_cnt(ci - NBUF))
                qev = qe[l][:].rearrange("p (a c) -> p a c", c=C)
                qov = qo[l][:].rearrange("p (a c) -> p a c", c=C)
                ov = outt[l][:].rearrange("p (t a u c) -> p t a u c", t=2, u=2, c=C)
                vec.scalar_tensor_tensor(
                    ov[:, 0, :, 0, :], qev[:, 1 : NW + 1, :], 3.0,
                    qev[:, 0:NW, :], _MUL, _ADD,
                ).then_inc(s_dve, 1)
                vec.scalar_tensor_tensor(
                    ov[:, 0, :, 1, :], qev[:, 1 : NW + 1, :], 3.0,
                    qev[:, 2 : NW + 2, :], _MUL, _ADD,
                ).then_inc(s_dve, 1)
                vec.wait_ge(s_cp, 2 * ci + 2)
                vec.scalar_tensor_tensor(
                    ov[:, 1, :, 0, :], qov[:, 1 : NW + 1, :], 3.0,
                    qov[:, 0:NW, :], _MUL, _ADD,
                ).then_inc(s_dve, 1)
                vec.scalar_tensor_tensor(
                    ov[:, 1, :, 1, :], qov[:, 1 : NW + 1, :], 3.0,
                    qov[:, 2 : NW + 2, :], _MUL, _ADD,
                ).then_inc(s_dve, 1)
            vec.sem_inc(s_fin, 1)

    return nc


_NC = None


def kernel(inputs: np.ndarray) -> np.ndarray:
    global _NC
    assert inputs.shape == (B, H, W, C), inputs.shape
    x = np.ascontiguousarray(inputs, dtype=np.float32).reshape(B, H, WC)
    if _NC is None:
        _NC = _build()
    we, wo = h_weights()
    in_maps = [
        {"x": x[i * BS : (i + 1) * BS], "we": we, "wo": wo} for i in range(NCORES)
    ]
    res = run_bass_kernel_spmd(_NC, in_maps, list(range(NCORES))).results
    out = np.empty((B, 2 * H, 2 * W, C), dtype=np.float32)
    for i in range(NCORES):
        out[i * BS : (i + 1) * BS] = res[i]["y"].reshape(BS, 2 * H, 2 * W, C)
    return out
